# revision 17
# baseline (speedup 1.0000x reference)
"""Trainium2 Bass kernel for nn_CliffordEngine (8-core data-parallel over batch).

Model: 4 passes of (grouped causal 3x3x3 conv over 32^3 grid -> tanh ->
8x8 cross-field mix -> alpha blend), then a sigmoid gate vs the original
state.  B=16, F=8 fields, 8 multivector components, G=32.

Mapping: each core takes 2 batch elements.  SBUF layout: partitions =
(b2, f8, c8) = 128; free dim = causally padded 34^3 volume stored in
fp8e4 (state scaled by S_X).  The conv runs as fp8 DoubleRow matmuls:
each instruction contracts TWO taps at once (K=256) using a strided
3D access pattern [128, 2(pair-delta), N] over the padded volume, with
contiguous-free subtiles of 11/11/10 rows x 34 cols per z-plane (the
2 wrap columns per row are computed and discarded).  Weights are fp8
(scaled by S_W); tanh de-scales via the activation scale; the mix is a
bf16 matmul whose weights fold in (1-alpha) and S_X so the DVE blend
writes the next fp8 state directly.
"""

import numpy as np

import concourse.bacc as bacc
import concourse.tile as tile
from concourse import mybir
from concourse.ap import AP
from concourse.bass_utils import run_bass_kernel_spmd

# Cl(3,0) Cayley table, transcribed from the reference model.
_TABLE = [
    [(0, 0, 1), (1, 1, 1), (2, 2, 1), (3, 3, 1), (4, 4, -1), (5, 5, -1), (6, 6, -1), (7, 7, -1)],
    [(1, 0, 1), (0, 1, 1), (4, 2, -1), (5, 3, -1), (2, 4, 1), (3, 5, 1), (7, 6, -1), (6, 7, -1)],
    [(2, 0, 1), (4, 1, 1), (0, 2, 1), (6, 3, -1), (1, 4, -1), (7, 5, 1), (3, 6, 1), (5, 7, 1)],
    [(3, 0, 1), (5, 1, 1), (6, 2, 1), (0, 3, 1), (7, 4, -1), (1, 5, -1), (2, 6, -1), (4, 7, -1)],
    [(4, 0, 1), (2, 1, 1), (1, 2, -1), (7, 3, 1), (0, 4, 1), (6, 5, -1), (5, 6, 1), (3, 7, 1)],
    [(5, 0, 1), (3, 1, 1), (7, 2, -1), (1, 3, -1), (6, 4, 1), (0, 5, 1), (4, 6, -1), (2, 7, -1)],
    [(6, 0, 1), (7, 1, 1), (3, 2, 1), (2, 3, -1), (5, 4, -1), (4, 5, 1), (0, 6, 1), (1, 7, 1)],
    [(7, 0, 1), (6, 1, 1), (5, 2, -1), (4, 3, 1), (3, 4, 1), (2, 5, -1), (1, 6, 1), (0, 7, 1)],
]

B, F, C, G = 16, 8, 8, 32
P_PASSES = 4
NCORES = 8
BL = B // NCORES            # local batch per core = 2
NPART = BL * F * C          # 128
GP = G + 2                  # padded extent 34
G3 = G * G * G
NTAPS = 27
ROWST = GP                  # 34
PLANEST = GP * GP           # 1156
PITCH = GP * GP * GP + 128  # padded volume + tail slop for pair reads
S_X = 16.0                  # fp8 state scale
S_W = 128.0                 # fp8 conv weight scale
HEIGHTS = [(0, 11), (11, 11), (22, 10)]   # (y0, rows) subtiles per plane
TAPS = [(kd, kh, kw) for kd in range(3) for kh in range(3) for kw in range(3)]

F32 = mybir.dt.float32
BF16 = mybir.dt.bfloat16
FP8 = mybir.dt.float8e4
NP_FP8 = mybir.dt.np(FP8)
NP_BF16 = mybir.dt.np(BF16)


def _softmax(x, axis):
    m = np.max(x, axis=axis, keepdims=True)
    e = np.exp(x - m)
    return e / np.sum(e, axis=axis, keepdims=True)


def _host_params(all_weights, all_biases, field_mix_logits, pass_alpha_logit,
                 gate_weight, gate_bias):
    """Precompute device weight tensors (runtime data, not baked into the NEFF)."""
    T = np.zeros((8, 8, 8), np.float32)
    for i, row in enumerate(_TABLE):
        for j, k, s in row:
            T[i, j, k] = s
    aw = np.asarray(all_weights, np.float32)          # [F, P, 27, 8]
    # W_eff[f,p,i,j,t] : out component i, in component j, tap t
    W = np.einsum('ijk,fpck->fpijc', T, aw)            # [F,P,8,8,27]
    mix = _softmax(np.asarray(field_mix_logits, np.float32), axis=2)  # [P,F,F] (g,f)
    alpha = 1.0 / (1.0 + np.exp(-np.asarray(pass_alpha_logit, np.float32)))  # [P]

    # conv lhsT: [k=(b,f,j), p, t, m=(b,f,i)] block-diagonal over (b, f), fp8
    convw = np.zeros((BL, F, C, P_PASSES, NTAPS, BL, F, C), np.float32)
    A = np.transpose(W, (0, 3, 1, 4, 2))               # [f, j, p, t, i]
    for b in range(BL):
        for f in range(F):
            convw[b, f, :, :, :, b, f, :] = A[f]
    convw = convw.reshape(NPART, P_PASSES, NTAPS, NPART)
    convw8 = np.clip(convw * S_W, -240.0, 240.0).astype(NP_FP8)

    # mix lhsT: [k=(b,f,ci), p, m=(b,g,co)] = S_X (1-alpha_p) mix[p,g,f] delta_b delta_c
    mixw = np.zeros((BL, F, C, P_PASSES, BL, F, C), np.float32)
    for b in range(BL):
        for p in range(P_PASSES):
            M2 = S_X * (1.0 - alpha[p]) * mix[p].T     # [f, g]
            for ci in range(C):
                mixw[b, :, ci, p, b, :, ci] = M2
    mixw = mixw.reshape(NPART, P_PASSES, NPART).astype(NP_BF16)

    # per-partition scalars: cols 0-3 bias_p, 4-7 alpha_p, 8 = -gw, 9 = -gb,
    # 10 = alpha3/S_X, 11+p = S_W * diag of the folded center tap of pass p
    pvec = np.zeros((BL, F, C, 16), np.float32)
    pvec = pvec.reshape(NPART, 16)
    ab = np.asarray(all_biases, np.float32).reshape(F, P_PASSES, C)
    pv4 = pvec.reshape(BL, F, C, 16)
    for p in range(P_PASSES):
        pv4[:, :, :, p] = ab[None, :, p, :]
        pv4[:, :, :, 4 + p] = alpha[p]
    pv4[:, :, :, 8] = -np.asarray(gate_weight, np.float32)[None]
    pv4[:, :, :, 9] = -np.asarray(gate_bias, np.float32)[None]
    pv4[:, :, :, 10] = alpha[P_PASSES - 1] / S_X
    # last pass's mix output feeds the fused gate directly (f32, unscaled)
    mixw[:, P_PASSES - 1] = (mixw[:, P_PASSES - 1].astype(np.float32)
                             / S_X).astype(NP_BF16)

    # active taps per pass; fold one diagonal tap to DVE when the count is odd
    active, fold = [], []
    for p in range(P_PASSES):
        nz = [t for t in range(NTAPS) if np.any(convw[:, p, t, :] != 0)]
        if not nz:
            nz = [0]
        ft = None
        if len(nz) % 2 == 1 and len(nz) > 1:
            for t in reversed(nz):
                blk = convw[:, p, t, :]
                if not np.any(blk[~np.eye(NPART, dtype=bool)]):
                    ft = t
                    break
            if ft is not None:
                nz.remove(ft)
                pvec[:, 11 + p] = S_W * np.diag(convw[:, p, ft, :])
        active.append(tuple(nz))
        fold.append(ft)
    return convw8, mixw, pvec, tuple(active), tuple(fold)


def build_nc(repeat_passes=1, active_taps=None, fold_taps=None,
             strided_rhs=True):
    """Build the per-core Bass program.  repeat_passes>1 is a timing variant.

    active_taps: optional per-pass tuple of tap indices to emit (taps whose
    weights are identically zero can be skipped exactly).  fold_taps: per-pass
    tap index whose (diagonal) contribution is added on DVE instead of a
    matmul, or None.
    """
    if active_taps is None:
        active_taps = tuple(tuple(range(NTAPS)) for _ in range(P_PASSES))
    if fold_taps is None:
        fold_taps = (None,) * P_PASSES
    # strided_rhs: 4D DoubleRow rhs AP skips the 2 wrap columns per row
    # (HW-validated; CoreSim only models the 3D contiguous form).
    nc = bacc.Bacc("TRN2", target_bir_lowering=False, debug=False,
                   num_devices=NCORES)
    state_in = nc.dram_tensor("state_in", [NPART, G3], F32, kind="ExternalInput")
    convw_d = nc.dram_tensor("convw", [NPART, P_PASSES, NTAPS, NPART], FP8,
                             kind="ExternalInput")
    mixw_d = nc.dram_tensor("mixw", [NPART, P_PASSES, NPART], BF16,
                            kind="ExternalInput")
    pvec_d = nc.dram_tensor("pvec", [NPART, 16], F32, kind="ExternalInput")
    out_d = nc.dram_tensor("out", [NPART, G3], F32, kind="ExternalOutput")

    # Persistent padded state buffers (ping/pong), fp8, flat with tail slop.
    ping = nc.alloc_sbuf_tensor("ping", [NPART, PITCH], FP8)
    pong = nc.alloc_sbuf_tensor("pong", [NPART, PITCH], FP8)

    def vol_ap(vol, off, dims):
        return AP(vol, off, [[PITCH, NPART]] + dims)

    with tile.TileContext(nc) as tc:
        with (
            tc.tile_pool(name="const", bufs=1) as constp,
            tc.tile_pool(name="stage", bufs=3) as stagep,
            tc.tile_pool(name="ytile", bufs=3) as yp,
            tc.tile_pool(name="gtile", bufs=6) as gp_,
            tc.tile_pool(name="psum1", bufs=5, space="PSUM") as ps1p,
            tc.tile_pool(name="psum2", bufs=3, space="PSUM") as ps2p,
        ):
            # memset only the halo/pad regions (interior is fully overwritten
            # by the init load / blends) so nothing serializes behind a full
            # 39KB-per-partition clear.
            for vol in (ping, pong):
                # planes 0-1, plus rows 0-1 of plane 2
                nc.gpsimd.memset(vol[:, 0:2 * PLANEST + 2 * ROWST + 2], 0)
                # rows 0-1 of planes 3..33 (strided over planes)
                nc.gpsimd.memset(
                    AP(vol, 3 * PLANEST, [[PITCH, NPART], [PLANEST, GP - 3],
                                          [1, 2 * ROWST + 2]]), 0)
                # cols 0-1 of every interior row (strided over planes x rows)
                nc.gpsimd.memset(
                    AP(vol, 2 * PLANEST + 2 * ROWST,
                       [[PITCH, NPART], [PLANEST, GP - 2], [ROWST, G], [1, 2]]),
                    0)
                # tail slop (overrun reads of the last pair subtile)
                nc.gpsimd.memset(vol[:, GP * PLANEST:], 0)

            convw_sb = constp.tile([NPART, P_PASSES, NTAPS, NPART], FP8,
                                   tag="convw")
            for p in range(P_PASSES):  # split so pass 0 weights land first
                nc.sync.dma_start(convw_sb[:, p], convw_d[:, p])
            mixw_sb = constp.tile([NPART, P_PASSES, NPART], BF16, tag="mixw")
            nc.sync.dma_start(mixw_sb[:], mixw_d[:])
            pvec_sb = constp.tile([NPART, 16], F32, tag="pvec")
            nc.sync.dma_start(pvec_sb[:], pvec_d[:])

            # initial load: f32 planes -> fp8 (x S_X) padded interior
            for z in range(G):
                st = stagep.tile([NPART, G * G], F32, tag="stage")
                nc.sync.dma_start(st[:], state_in[:, z * 1024:(z + 1) * 1024])
                nc.scalar.activation(
                    out=vol_ap(ping, (z + 2) * PLANEST + 2 * ROWST + 2,
                               [[ROWST, G], [1, G]]),
                    in_=st[:].rearrange("p (a b) -> p a b", a=G),
                    func=mybir.ActivationFunctionType.Copy, scale=S_X)

            def sub_ap(base_ap, off, dims):
                return AP(base_ap.tensor, base_ap.offset + off,
                          [list(base_ap.ap[0])] + dims)

            def conv_lhsT(p, tA, tB):
                off = (p * NTAPS + tA) * NPART
                return sub_ap(convw_sb[:], off,
                              [[(tB - tA) * NPART, 2], [1, NPART]])

            cur, nxt = ping, pong
            for rp in range(repeat_passes):
                for p in range(P_PASSES):
                    act = active_taps[p]
                    pairs = [(act[2 * i], act[2 * i + 1])
                             for i in range(len(act) // 2)]
                    single = act[-1] if len(act) % 2 else None
                    ngroups = len(pairs) + (1 if single is not None else 0)
                    for z0 in range(G):
                        for (y0, hh) in HEIGHTS:
                            if strided_rhs:
                                ps1 = ps1p.tile([NPART, hh, G], F32,
                                                space="PSUM", tag="ps1")
                            else:
                                ps1 = ps1p.tile([NPART, hh * ROWST], F32,
                                                space="PSUM", tag="ps1")
                            gi = 0
                            for (tA, tB) in pairs:
                                kdA, khA, kwA = TAPS[tA]
                                kdB, khB, kwB = TAPS[tB]
                                offA = (z0 + kdA) * PLANEST + (y0 + khA) * ROWST + kwA
                                offB = (z0 + kdB) * PLANEST + (y0 + khB) * ROWST + kwB
                                if strided_rhs:
                                    rhs = vol_ap(cur, offA,
                                                 [[offB - offA, 2],
                                                  [ROWST, hh], [1, G]])
                                else:
                                    rhs = vol_ap(cur, offA,
                                                 [[offB - offA, 2],
                                                  [1, hh * ROWST]])
                                nc.tensor.matmul(
                                    out=ps1[:], lhsT=conv_lhsT(p, tA, tB),
                                    rhs=rhs, start=(gi == 0),
                                    stop=(gi == ngroups - 1),
                                    perf_mode=mybir.MatmulPerfMode.DoubleRow)
                                gi += 1
                            if single is not None:
                                kd, kh, kw = TAPS[single]
                                off = (z0 + kd) * PLANEST + (y0 + kh) * ROWST + kw
                                if strided_rhs:
                                    rhs = vol_ap(cur, off, [[ROWST, hh], [1, G]])
                                else:
                                    rhs = vol_ap(cur, off, [[1, hh * ROWST]])
                                nc.tensor.matmul(
                                    out=ps1[:], lhsT=convw_sb[:, p, single],
                                    rhs=rhs, start=(gi == 0),
                                    stop=(gi == ngroups - 1))
                                gi += 1
                            # tanh over the valid columns of psum
                            y = yp.tile([NPART, hh, G], BF16, tag="y")
                            if strided_rhs:
                                ps1v = ps1[:]
                            else:
                                ps1v = sub_ap(ps1[:], 0, [[ROWST, hh], [1, G]])
                            tanh_in = ps1v
                            if fold_taps[p] is not None:
                                kd, kh, kw = TAPS[fold_taps[p]]
                                coff = (z0 + kd) * PLANEST + (y0 + kh) * ROWST + kw
                                tmp = yp.tile([NPART, hh, G], F32, tag="tmp")
                                nc.vector.scalar_tensor_tensor(
                                    out=tmp[:],
                                    in0=vol_ap(cur, coff, [[ROWST, hh], [1, G]]),
                                    scalar=pvec_sb[:, 11 + p:12 + p],
                                    in1=ps1v,
                                    op0=mybir.AluOpType.mult,
                                    op1=mybir.AluOpType.add,
                                )
                                tanh_in = tmp[:]
                            nc.scalar.activation(
                                out=y[:], in_=tanh_in,
                                func=mybir.ActivationFunctionType.Tanh,
                                bias=pvec_sb[:, p:p + 1], scale=1.0 / (S_X * S_W),
                            )
                            ps2 = ps2p.tile([NPART, hh, G], F32, space="PSUM",
                                            tag="ps2")
                            nc.tensor.matmul(out=ps2[:], lhsT=mixw_sb[:, p],
                                             rhs=y[:], start=True, stop=True)
                            intr = (z0 + 2) * PLANEST + (y0 + 2) * ROWST + 2
                            last = (rp == repeat_passes - 1 and
                                    p == P_PASSES - 1)
                            if not last:
                                # nxt = alpha * cur + ps2  (S_X units, fp8)
                                nc.vector.scalar_tensor_tensor(
                                    out=vol_ap(nxt, intr, [[ROWST, hh], [1, G]]),
                                    in0=vol_ap(cur, intr, [[ROWST, hh], [1, G]]),
                                    scalar=pvec_sb[:, 4 + p:5 + p],
                                    in1=ps2[:],
                                    op0=mybir.AluOpType.mult,
                                    op1=mybir.AluOpType.add,
                                )
                                continue
                            # fused gate (last pass): ps2 here is unscaled, so
                            # x4 = (alpha3/S_X) * cur + ps2;
                            # out = old + sigmoid(-(gw*old+gb)) * (x4 - old)
                            sl = slice(z0 * 1024 + y0 * G, z0 * 1024 + (y0 + hh) * G)
                            old = gp_.tile([NPART, hh, G], F32, tag="old")
                            nc.sync.dma_start(
                                old[:], state_in[:, sl].rearrange(
                                    "p (a b) -> p a b", a=hh))
                            hg = gp_.tile([NPART, hh, G], F32, tag="hg")
                            nc.scalar.activation(
                                out=hg[:], in_=old[:],
                                func=mybir.ActivationFunctionType.Sigmoid,
                                bias=pvec_sb[:, 9:10], scale=pvec_sb[:, 8:9],
                            )
                            x4 = gp_.tile([NPART, hh, G], F32, tag="x4")
                            nc.vector.scalar_tensor_tensor(
                                out=x4[:],
                                in0=vol_ap(cur, intr, [[ROWST, hh], [1, G]]),
                                scalar=pvec_sb[:, 10:11],
                                in1=ps2[:],
                                op0=mybir.AluOpType.mult,
                                op1=mybir.AluOpType.add,
                            )
                            d = gp_.tile([NPART, hh, G], F32, tag="d")
                            nc.vector.tensor_tensor(
                                out=d[:], in0=x4[:], in1=old[:],
                                op=mybir.AluOpType.subtract)
                            nc.vector.tensor_tensor(
                                out=d[:], in0=hg[:], in1=d[:],
                                op=mybir.AluOpType.mult)
                            o = gp_.tile([NPART, hh, G], F32, tag="o")
                            nc.vector.tensor_tensor(
                                out=o[:], in0=old[:], in1=d[:],
                                op=mybir.AluOpType.add)
                            nc.sync.dma_start(
                                out_d[:, sl],
                                o[:].rearrange("p a b -> p (a b)"))
                    cur, nxt = nxt, cur

    nc.compile()
    return nc


_NC_CACHE = {}


def _get_nc(repeat_passes=1, active_taps=None, fold_taps=None,
            strided_rhs=True):
    key = (repeat_passes, active_taps, fold_taps, strided_rhs)
    if key not in _NC_CACHE:
        _NC_CACHE[key] = build_nc(repeat_passes, active_taps, fold_taps,
                                  strided_rhs)
    return _NC_CACHE[key]


def make_in_maps(state, all_weights, all_biases, field_mix_logits,
                 pass_alpha_logit, gate_weight, gate_bias):
    convw8, mixw, pvec, active, fold = _host_params(
        all_weights, all_biases, field_mix_logits, pass_alpha_logit,
        gate_weight, gate_bias)
    state = np.ascontiguousarray(np.asarray(state, np.float32))
    in_maps = []
    for i in range(NCORES):
        shard = state[BL * i:BL * (i + 1)].reshape(NPART, G3)
        in_maps.append({
            "state_in": shard,
            "convw": convw8,
            "mixw": mixw,
            "pvec": pvec,
        })
    return in_maps, active, fold


def kernel(state, all_weights, all_biases, field_mix_logits,
           pass_alpha_logit, gate_weight, gate_bias):
    in_maps, active, fold = make_in_maps(state, all_weights, all_biases,
                                         field_mix_logits, pass_alpha_logit,
                                         gate_weight, gate_bias)
    nc = _get_nc(1, active, fold)
    for attempt in range(5):
        try:
            res = run_bass_kernel_spmd(nc, in_maps, core_ids=list(range(NCORES)))
            break
        except Exception:  # transient device-recovery errors
            if attempt == 4:
                raise
            import time as _time
            _time.sleep(10.0 * (attempt + 1))
    outs = [res.results[i]["out"].reshape(BL, F, C, G, G, G)
            for i in range(NCORES)]
    return np.concatenate(outs, axis=0).astype(np.float32)


# revision 18
# speedup vs baseline: 1.0131x; 1.0131x over previous
"""Trainium2 Bass kernel for nn_CliffordEngine (8-core data-parallel over batch).

Model: 4 passes of (grouped causal 3x3x3 conv over 32^3 grid -> tanh ->
8x8 cross-field mix -> alpha blend), then a sigmoid gate vs the original
state.  B=16, F=8 fields, 8 multivector components, G=32.

Mapping: each core takes 2 batch elements.  SBUF layout: partitions =
(b2, f8, c8) = 128; free dim = causally padded 34^3 volume stored in
fp8e4 (state scaled by S_X).  The conv runs as fp8 DoubleRow matmuls:
each instruction contracts TWO taps at once (K=256) using a strided
3D access pattern [128, 2(pair-delta), N] over the padded volume, with
contiguous-free subtiles of 11/11/10 rows x 34 cols per z-plane (the
2 wrap columns per row are computed and discarded).  Weights are fp8
(scaled by S_W); tanh de-scales via the activation scale; the mix is a
bf16 matmul whose weights fold in (1-alpha) and S_X so the DVE blend
writes the next fp8 state directly.
"""

import numpy as np

import concourse.bacc as bacc
import concourse.tile as tile
from concourse import mybir
from concourse.ap import AP
from concourse.bass_utils import run_bass_kernel_spmd

# Cl(3,0) Cayley table, transcribed from the reference model.
_TABLE = [
    [(0, 0, 1), (1, 1, 1), (2, 2, 1), (3, 3, 1), (4, 4, -1), (5, 5, -1), (6, 6, -1), (7, 7, -1)],
    [(1, 0, 1), (0, 1, 1), (4, 2, -1), (5, 3, -1), (2, 4, 1), (3, 5, 1), (7, 6, -1), (6, 7, -1)],
    [(2, 0, 1), (4, 1, 1), (0, 2, 1), (6, 3, -1), (1, 4, -1), (7, 5, 1), (3, 6, 1), (5, 7, 1)],
    [(3, 0, 1), (5, 1, 1), (6, 2, 1), (0, 3, 1), (7, 4, -1), (1, 5, -1), (2, 6, -1), (4, 7, -1)],
    [(4, 0, 1), (2, 1, 1), (1, 2, -1), (7, 3, 1), (0, 4, 1), (6, 5, -1), (5, 6, 1), (3, 7, 1)],
    [(5, 0, 1), (3, 1, 1), (7, 2, -1), (1, 3, -1), (6, 4, 1), (0, 5, 1), (4, 6, -1), (2, 7, -1)],
    [(6, 0, 1), (7, 1, 1), (3, 2, 1), (2, 3, -1), (5, 4, -1), (4, 5, 1), (0, 6, 1), (1, 7, 1)],
    [(7, 0, 1), (6, 1, 1), (5, 2, -1), (4, 3, 1), (3, 4, 1), (2, 5, -1), (1, 6, 1), (0, 7, 1)],
]

B, F, C, G = 16, 8, 8, 32
P_PASSES = 4
NCORES = 8
BL = B // NCORES            # local batch per core = 2
NPART = BL * F * C          # 128
GP = G + 2                  # padded extent 34
G3 = G * G * G
NTAPS = 27
ROWST = GP                  # 34
PLANEST = GP * GP           # 1156
PITCH = GP * GP * GP + 128  # padded volume + tail slop for pair reads
S_X = 16.0                  # fp8 state scale
S_W = 128.0                 # fp8 conv weight scale
HEIGHTS = [(0, 16), (16, 16)]   # (y0, rows) subtiles per plane
TAPS = [(kd, kh, kw) for kd in range(3) for kh in range(3) for kw in range(3)]

F32 = mybir.dt.float32
BF16 = mybir.dt.bfloat16
FP8 = mybir.dt.float8e4
NP_FP8 = mybir.dt.np(FP8)
NP_BF16 = mybir.dt.np(BF16)


def _softmax(x, axis):
    m = np.max(x, axis=axis, keepdims=True)
    e = np.exp(x - m)
    return e / np.sum(e, axis=axis, keepdims=True)


def _host_params(all_weights, all_biases, field_mix_logits, pass_alpha_logit,
                 gate_weight, gate_bias):
    """Precompute device weight tensors (runtime data, not baked into the NEFF)."""
    T = np.zeros((8, 8, 8), np.float32)
    for i, row in enumerate(_TABLE):
        for j, k, s in row:
            T[i, j, k] = s
    aw = np.asarray(all_weights, np.float32)          # [F, P, 27, 8]
    # W_eff[f,p,i,j,t] : out component i, in component j, tap t
    W = np.einsum('ijk,fpck->fpijc', T, aw)            # [F,P,8,8,27]
    mix = _softmax(np.asarray(field_mix_logits, np.float32), axis=2)  # [P,F,F] (g,f)
    alpha = 1.0 / (1.0 + np.exp(-np.asarray(pass_alpha_logit, np.float32)))  # [P]

    # conv lhsT: [k=(b,f,j), p, t, m=(b,f,i)] block-diagonal over (b, f), fp8
    convw = np.zeros((BL, F, C, P_PASSES, NTAPS, BL, F, C), np.float32)
    A = np.transpose(W, (0, 3, 1, 4, 2))               # [f, j, p, t, i]
    for b in range(BL):
        for f in range(F):
            convw[b, f, :, :, :, b, f, :] = A[f]
    convw = convw.reshape(NPART, P_PASSES, NTAPS, NPART)
    convw8 = np.clip(convw * S_W, -240.0, 240.0).astype(NP_FP8)

    # mix lhsT: [k=(b,f,ci), p, m=(b,g,co)] = S_X (1-alpha_p) mix[p,g,f] delta_b delta_c
    mixw = np.zeros((BL, F, C, P_PASSES, BL, F, C), np.float32)
    for b in range(BL):
        for p in range(P_PASSES):
            M2 = S_X * (1.0 - alpha[p]) * mix[p].T     # [f, g]
            for ci in range(C):
                mixw[b, :, ci, p, b, :, ci] = M2
    mixw = mixw.reshape(NPART, P_PASSES, NPART).astype(NP_BF16)

    # per-partition scalars: cols 0-3 bias_p, 4-7 alpha_p, 8 = -gw, 9 = -gb,
    # 10 = alpha3/S_X, 11+p = S_W * diag of the folded center tap of pass p
    pvec = np.zeros((BL, F, C, 16), np.float32)
    pvec = pvec.reshape(NPART, 16)
    ab = np.asarray(all_biases, np.float32).reshape(F, P_PASSES, C)
    pv4 = pvec.reshape(BL, F, C, 16)
    for p in range(P_PASSES):
        pv4[:, :, :, p] = ab[None, :, p, :]
        pv4[:, :, :, 4 + p] = alpha[p]
    pv4[:, :, :, 8] = -np.asarray(gate_weight, np.float32)[None]
    pv4[:, :, :, 9] = -np.asarray(gate_bias, np.float32)[None]
    pv4[:, :, :, 10] = alpha[P_PASSES - 1] / S_X
    # last pass's mix output feeds the fused gate directly (f32, unscaled)
    mixw[:, P_PASSES - 1] = (mixw[:, P_PASSES - 1].astype(np.float32)
                             / S_X).astype(NP_BF16)

    # active taps per pass; fold one diagonal tap to DVE when the count is odd
    active, fold = [], []
    for p in range(P_PASSES):
        nz = [t for t in range(NTAPS) if np.any(convw[:, p, t, :] != 0)]
        if not nz:
            nz = [0]
        ft = None
        if len(nz) % 2 == 1 and len(nz) > 1:
            for t in reversed(nz):
                blk = convw[:, p, t, :]
                if not np.any(blk[~np.eye(NPART, dtype=bool)]):
                    ft = t
                    break
            if ft is not None:
                nz.remove(ft)
                pvec[:, 11 + p] = S_W * np.diag(convw[:, p, ft, :])
        active.append(tuple(nz))
        fold.append(ft)
    return convw8, mixw, pvec, tuple(active), tuple(fold)


def build_nc(repeat_passes=1, active_taps=None, fold_taps=None,
             strided_rhs=True):
    """Build the per-core Bass program.  repeat_passes>1 is a timing variant.

    active_taps: optional per-pass tuple of tap indices to emit (taps whose
    weights are identically zero can be skipped exactly).  fold_taps: per-pass
    tap index whose (diagonal) contribution is added on DVE instead of a
    matmul, or None.
    """
    if active_taps is None:
        active_taps = tuple(tuple(range(NTAPS)) for _ in range(P_PASSES))
    if fold_taps is None:
        fold_taps = (None,) * P_PASSES
    # strided_rhs: 4D DoubleRow rhs AP skips the 2 wrap columns per row
    # (HW-validated; CoreSim only models the 3D contiguous form).
    nc = bacc.Bacc("TRN2", target_bir_lowering=False, debug=False,
                   num_devices=NCORES)
    state_in = nc.dram_tensor("state_in", [NPART, G3], F32, kind="ExternalInput")
    convw_d = nc.dram_tensor("convw", [NPART, P_PASSES, NTAPS, NPART], FP8,
                             kind="ExternalInput")
    mixw_d = nc.dram_tensor("mixw", [NPART, P_PASSES, NPART], BF16,
                            kind="ExternalInput")
    pvec_d = nc.dram_tensor("pvec", [NPART, 16], F32, kind="ExternalInput")
    out_d = nc.dram_tensor("out", [NPART, G3], F32, kind="ExternalOutput")

    # Persistent padded state buffers (ping/pong), fp8, flat with tail slop.
    ping = nc.alloc_sbuf_tensor("ping", [NPART, PITCH], FP8)
    pong = nc.alloc_sbuf_tensor("pong", [NPART, PITCH], FP8)

    def vol_ap(vol, off, dims):
        return AP(vol, off, [[PITCH, NPART]] + dims)

    with tile.TileContext(nc) as tc:
        with (
            tc.tile_pool(name="const", bufs=1) as constp,
            tc.tile_pool(name="stage", bufs=3) as stagep,
            tc.tile_pool(name="ytile", bufs=3) as yp,
            tc.tile_pool(name="gtile", bufs=3) as gp_,
            tc.tile_pool(name="psum1", bufs=5, space="PSUM") as ps1p,
            tc.tile_pool(name="psum2", bufs=3, space="PSUM") as ps2p,
        ):
            # memset only the halo/pad regions (interior is fully overwritten
            # by the init load / blends) so nothing serializes behind a full
            # 39KB-per-partition clear.
            for vol in (ping, pong):
                # planes 0-1, plus rows 0-1 of plane 2
                nc.gpsimd.memset(vol[:, 0:2 * PLANEST + 2 * ROWST + 2], 0)
                # rows 0-1 of planes 3..33 (strided over planes)
                nc.gpsimd.memset(
                    AP(vol, 3 * PLANEST, [[PITCH, NPART], [PLANEST, GP - 3],
                                          [1, 2 * ROWST + 2]]), 0)
                # cols 0-1 of every interior row (strided over planes x rows)
                nc.gpsimd.memset(
                    AP(vol, 2 * PLANEST + 2 * ROWST,
                       [[PITCH, NPART], [PLANEST, GP - 2], [ROWST, G], [1, 2]]),
                    0)
                # tail slop (overrun reads of the last pair subtile)
                nc.gpsimd.memset(vol[:, GP * PLANEST:], 0)

            convw_sb = constp.tile([NPART, P_PASSES, NTAPS, NPART], FP8,
                                   tag="convw")
            for p in range(P_PASSES):  # split so pass 0 weights land first
                nc.sync.dma_start(convw_sb[:, p], convw_d[:, p])
            mixw_sb = constp.tile([NPART, P_PASSES, NPART], BF16, tag="mixw")
            nc.sync.dma_start(mixw_sb[:], mixw_d[:])
            pvec_sb = constp.tile([NPART, 16], F32, tag="pvec")
            nc.sync.dma_start(pvec_sb[:], pvec_d[:])

            # initial load: f32 planes -> fp8 (x S_X) padded interior
            for z in range(G):
                st = stagep.tile([NPART, G * G], F32, tag="stage")
                nc.sync.dma_start(st[:], state_in[:, z * 1024:(z + 1) * 1024])
                nc.scalar.activation(
                    out=vol_ap(ping, (z + 2) * PLANEST + 2 * ROWST + 2,
                               [[ROWST, G], [1, G]]),
                    in_=st[:].rearrange("p (a b) -> p a b", a=G),
                    func=mybir.ActivationFunctionType.Copy, scale=S_X)

            def sub_ap(base_ap, off, dims):
                return AP(base_ap.tensor, base_ap.offset + off,
                          [list(base_ap.ap[0])] + dims)

            def conv_lhsT(p, tA, tB):
                off = (p * NTAPS + tA) * NPART
                return sub_ap(convw_sb[:], off,
                              [[(tB - tA) * NPART, 2], [1, NPART]])

            cur, nxt = ping, pong
            for rp in range(repeat_passes):
                for p in range(P_PASSES):
                    act = active_taps[p]
                    pairs = [(act[2 * i], act[2 * i + 1])
                             for i in range(len(act) // 2)]
                    single = act[-1] if len(act) % 2 else None
                    ngroups = len(pairs) + (1 if single is not None else 0)
                    for z0 in range(G):
                        for (y0, hh) in HEIGHTS:
                            if strided_rhs:
                                ps1 = ps1p.tile([NPART, hh, G], F32,
                                                space="PSUM", tag="ps1")
                            else:
                                ps1 = ps1p.tile([NPART, hh * ROWST], F32,
                                                space="PSUM", tag="ps1")
                            gi = 0
                            for (tA, tB) in pairs:
                                kdA, khA, kwA = TAPS[tA]
                                kdB, khB, kwB = TAPS[tB]
                                offA = (z0 + kdA) * PLANEST + (y0 + khA) * ROWST + kwA
                                offB = (z0 + kdB) * PLANEST + (y0 + khB) * ROWST + kwB
                                if strided_rhs:
                                    rhs = vol_ap(cur, offA,
                                                 [[offB - offA, 2],
                                                  [ROWST, hh], [1, G]])
                                else:
                                    rhs = vol_ap(cur, offA,
                                                 [[offB - offA, 2],
                                                  [1, hh * ROWST]])
                                nc.tensor.matmul(
                                    out=ps1[:], lhsT=conv_lhsT(p, tA, tB),
                                    rhs=rhs, start=(gi == 0),
                                    stop=(gi == ngroups - 1),
                                    perf_mode=mybir.MatmulPerfMode.DoubleRow)
                                gi += 1
                            if single is not None:
                                kd, kh, kw = TAPS[single]
                                off = (z0 + kd) * PLANEST + (y0 + kh) * ROWST + kw
                                if strided_rhs:
                                    rhs = vol_ap(cur, off, [[ROWST, hh], [1, G]])
                                else:
                                    rhs = vol_ap(cur, off, [[1, hh * ROWST]])
                                nc.tensor.matmul(
                                    out=ps1[:], lhsT=convw_sb[:, p, single],
                                    rhs=rhs, start=(gi == 0),
                                    stop=(gi == ngroups - 1))
                                gi += 1
                            # tanh over the valid columns of psum
                            y = yp.tile([NPART, hh, G], BF16, tag="y")
                            if strided_rhs:
                                ps1v = ps1[:]
                            else:
                                ps1v = sub_ap(ps1[:], 0, [[ROWST, hh], [1, G]])
                            tanh_in = ps1v
                            if fold_taps[p] is not None:
                                kd, kh, kw = TAPS[fold_taps[p]]
                                coff = (z0 + kd) * PLANEST + (y0 + kh) * ROWST + kw
                                tmp = yp.tile([NPART, hh, G], F32, tag="tmp")
                                nc.vector.scalar_tensor_tensor(
                                    out=tmp[:],
                                    in0=vol_ap(cur, coff, [[ROWST, hh], [1, G]]),
                                    scalar=pvec_sb[:, 11 + p:12 + p],
                                    in1=ps1v,
                                    op0=mybir.AluOpType.mult,
                                    op1=mybir.AluOpType.add,
                                )
                                tanh_in = tmp[:]
                            nc.scalar.activation(
                                out=y[:], in_=tanh_in,
                                func=mybir.ActivationFunctionType.Tanh,
                                bias=pvec_sb[:, p:p + 1], scale=1.0 / (S_X * S_W),
                            )
                            ps2 = ps2p.tile([NPART, hh, G], F32, space="PSUM",
                                            tag="ps2")
                            nc.tensor.matmul(out=ps2[:], lhsT=mixw_sb[:, p],
                                             rhs=y[:], start=True, stop=True)
                            intr = (z0 + 2) * PLANEST + (y0 + 2) * ROWST + 2
                            last = (rp == repeat_passes - 1 and
                                    p == P_PASSES - 1)
                            if not last:
                                # nxt = alpha * cur + ps2  (S_X units, fp8)
                                nc.vector.scalar_tensor_tensor(
                                    out=vol_ap(nxt, intr, [[ROWST, hh], [1, G]]),
                                    in0=vol_ap(cur, intr, [[ROWST, hh], [1, G]]),
                                    scalar=pvec_sb[:, 4 + p:5 + p],
                                    in1=ps2[:],
                                    op0=mybir.AluOpType.mult,
                                    op1=mybir.AluOpType.add,
                                )
                                continue
                            # fused gate (last pass): ps2 here is unscaled, so
                            # x4 = (alpha3/S_X) * cur + ps2;
                            # out = old + sigmoid(-(gw*old+gb)) * (x4 - old)
                            sl = slice(z0 * 1024 + y0 * G, z0 * 1024 + (y0 + hh) * G)
                            old = gp_.tile([NPART, hh, G], F32, tag="old")
                            nc.sync.dma_start(
                                old[:], state_in[:, sl].rearrange(
                                    "p (a b) -> p a b", a=hh))
                            hg = gp_.tile([NPART, hh, G], F32, tag="hg")
                            nc.scalar.activation(
                                out=hg[:], in_=old[:],
                                func=mybir.ActivationFunctionType.Sigmoid,
                                bias=pvec_sb[:, 9:10], scale=pvec_sb[:, 8:9],
                            )
                            x4 = gp_.tile([NPART, hh, G], F32, tag="x4")
                            nc.vector.scalar_tensor_tensor(
                                out=x4[:],
                                in0=vol_ap(cur, intr, [[ROWST, hh], [1, G]]),
                                scalar=pvec_sb[:, 10:11],
                                in1=ps2[:],
                                op0=mybir.AluOpType.mult,
                                op1=mybir.AluOpType.add,
                            )
                            d = gp_.tile([NPART, hh, G], F32, tag="d")
                            nc.vector.tensor_tensor(
                                out=d[:], in0=x4[:], in1=old[:],
                                op=mybir.AluOpType.subtract)
                            nc.vector.tensor_tensor(
                                out=d[:], in0=hg[:], in1=d[:],
                                op=mybir.AluOpType.mult)
                            o = gp_.tile([NPART, hh, G], F32, tag="o")
                            nc.vector.tensor_tensor(
                                out=o[:], in0=old[:], in1=d[:],
                                op=mybir.AluOpType.add)
                            nc.sync.dma_start(
                                out_d[:, sl],
                                o[:].rearrange("p a b -> p (a b)"))
                    cur, nxt = nxt, cur

    nc.compile()
    return nc


_NC_CACHE = {}


def _get_nc(repeat_passes=1, active_taps=None, fold_taps=None,
            strided_rhs=True):
    key = (repeat_passes, active_taps, fold_taps, strided_rhs)
    if key not in _NC_CACHE:
        _NC_CACHE[key] = build_nc(repeat_passes, active_taps, fold_taps,
                                  strided_rhs)
    return _NC_CACHE[key]


def make_in_maps(state, all_weights, all_biases, field_mix_logits,
                 pass_alpha_logit, gate_weight, gate_bias):
    convw8, mixw, pvec, active, fold = _host_params(
        all_weights, all_biases, field_mix_logits, pass_alpha_logit,
        gate_weight, gate_bias)
    state = np.ascontiguousarray(np.asarray(state, np.float32))
    in_maps = []
    for i in range(NCORES):
        shard = state[BL * i:BL * (i + 1)].reshape(NPART, G3)
        in_maps.append({
            "state_in": shard,
            "convw": convw8,
            "mixw": mixw,
            "pvec": pvec,
        })
    return in_maps, active, fold


def kernel(state, all_weights, all_biases, field_mix_logits,
           pass_alpha_logit, gate_weight, gate_bias):
    in_maps, active, fold = make_in_maps(state, all_weights, all_biases,
                                         field_mix_logits, pass_alpha_logit,
                                         gate_weight, gate_bias)
    nc = _get_nc(1, active, fold)
    for attempt in range(5):
        try:
            res = run_bass_kernel_spmd(nc, in_maps, core_ids=list(range(NCORES)))
            break
        except Exception:  # transient device-recovery errors
            if attempt == 4:
                raise
            import time as _time
            _time.sleep(10.0 * (attempt + 1))
    outs = [res.results[i]["out"].reshape(BL, F, C, G, G, G)
            for i in range(NCORES)]
    return np.concatenate(outs, axis=0).astype(np.float32)
